# revision 10
# baseline (speedup 1.0000x reference)
"""Pairwise cosine similarity on 8 Trainium2 NeuronCores.

Computes sim[n, m] = <x_n, y_m> / max(||x_n|| * ||y_m||, eps) for
input1 [8192, 128], input2 [8192, 128] -> out [8192, 8192] (all fp32).

Sharding: input1 rows are split 8 ways (data parallel, 1024 rows/core);
input2 is replicated. Each core computes one [1024, 8192] output stripe;
the host concatenates stripes.

The kernel is HBM-store-bound (32 MiB of output per core vs ~27us of
matmul), so the schedule is built around keeping the Sync-HWDGE store
queue saturated at its ~400 GB/s packet-rate ceiling from as early in
the kernel as possible:

- The host feeds operands partition-major (x'[p*nbx+b] = x[b*128+p]),
  so every SBUF partition loads one contiguous DRAM run and load DMAs
  use KB-scale packets instead of one packet per 512B row.
- All loads are issued up front on ACT's HWDGE queue (separate FIFO
  from the store queue; nothing ever blocks a load issue), into
  persistent raw tiles.
- Rows of both inputs are L2-normalized in natural layout, PE-transposed
  into [d, rows] layout (rounded to fp32r), and each output stripe is a
  single matmul of the normalized operands. PSUM results stream through
  SBUF staging to DRAM in 2048-column chunks: 8 KiB store lines, the
  packet size at which the 16 DMA engines peak.

Note on eps: the reference divides by max(n1*n2, 1e-8). For these inputs
row norms are ~sqrt(128), so the eps clamp never binds and normalizing
each operand first is numerically equivalent (to fp32 rounding).
"""

import numpy as np

import concourse.bass as bass
import concourse.tile as tile
from concourse import bacc, masks, mybir
from concourse.bass_utils import run_bass_kernel_spmd

N_CORES = 8
D = 128          # feature dim == partition count
P = 128          # SBUF partitions
NT = 512         # matmul moving free dim (one fp32 PSUM bank)
OCHUNK = 2048    # output columns per staging buffer (8KB/partition, 1MiB DMA)
MMCOLS = 1024    # PSUM matmul tile columns (2 banks, 2 matmuls, 1 copy)

F32 = mybir.dt.float32
F16 = mybir.dt.float16
BF16 = mybir.dt.bfloat16
ACTF = mybir.ActivationFunctionType


def build_nc(rows_per_core: int, corpus_rows: int) -> bass.Bass:
    # Bacc (not raw Bass): its compile() pipeline splits multi-sem waits into
    # event-semaphore instructions, which self-loading fp32/fp32r matmuls
    # need (the ISA LDWEIGHTS struct can carry only one wait).
    nc = bacc.Bacc(None)

    x = nc.dram_tensor("x", [rows_per_core, D], F32, kind="ExternalInput")
    y = nc.dram_tensor("y", [corpus_rows, D], F32, kind="ExternalInput")
    out = nc.dram_tensor(
        "out", [rows_per_core, corpus_rows], F32, kind="ExternalOutput"
    )

    nbx = rows_per_core // P         # x row-blocks (8)
    nby = corpus_rows // P           # y row-blocks (64)

    with tile.TileContext(nc) as tc:
        with (
            tc.tile_pool(name="const", bufs=1) as constp,
            tc.tile_pool(name="persist", bufs=1) as persist,
            tc.tile_pool(name="yt", bufs=3) as ytp,
            tc.tile_pool(name="stat", bufs=6) as statp,
            tc.tile_pool(name="sq", bufs=5) as sqp,
            tc.tile_pool(name="n16", bufs=5) as n16p,
            tc.tile_pool(name="obuf", bufs=4) as obufp,
            tc.tile_pool(name="tp", bufs=2, space=bass.MemorySpace.PSUM) as tpsum,
            tc.tile_pool(name="mm", bufs=3, space=bass.MemorySpace.PSUM) as mpsum,
        ):
            ident = constp.tile([P, P], F16)
            masks.make_identity(nc, ident[:])

            # Partition-major DRAM layout (host pre-transposes the shards):
            # x row b*128+p lives at [p, b, :], so partition p's rows are one
            # contiguous nbx*512B run. Block b still holds rows {b*128+p}, so
            # xT/yT column order (and therefore the stores) stays natural.
            x_view = x[:].rearrange("(p b) d -> p b d", p=P)
            y_view = y[:].rearrange("(p b) d -> p b d", p=P)

            # Uniform max-size chunks: 8KB store lines are the packet size at
            # which the 16 DMA engines hit their ~400GB/s aggregate ceiling
            # (smaller chunks pay 112ns/packet overhead on less data).
            chunk_cols = [OCHUNK] * (corpus_rows // OCHUNK)
            rem = corpus_rows - sum(chunk_cols)
            if rem:
                chunk_cols.append(rem)
            chunk_starts = []
            s = 0
            for cols in chunk_cols:
                chunk_starts.append(s)
                s += cols

            # All loads issued up front on ACT's HWDGE queue: its FIFO holds
            # only dma_starts at this point, so every transfer is in flight
            # within the first microseconds regardless of compute stalls.
            # (SWDGE's GpSimd firmware needs ~9.5us for its first packet;
            # the Sync FIFO would head-of-line-block stores behind loads.)
            xraw = persist.tile([P, nbx, D], F32)
            yraw = persist.tile([P, nby, D], F32)
            # x + first y chunk ride the Sync queue: Sync clears its startup
            # barrier ~2us before ACT, and only 1.5MB sits ahead of the first
            # store descriptor. Later chunks go on ACT's queue so the store
            # stream never queues behind them.
            nc.sync.dma_start(out=xraw[:], in_=x_view[:, :nbx, :])
            for c, cols in enumerate(chunk_cols):
                b0 = chunk_starts[c] // P
                bcnt = cols // P
                eng = nc.sync if c == 0 else nc.scalar
                eng.dma_start(
                    out=yraw[:, b0 : b0 + bcnt, :],
                    in_=y_view[:, b0 : b0 + bcnt, :],
                )

            # PE warm-up: dummy bf16 matmuls overlapping the initial load so
            # the HAM clock gate opens (1.2 -> 2.4 GHz) before the first real
            # matmul.
            wt = constp.tile([P, NT], BF16)
            nc.gpsimd.memset(wt[:], 0.0)
            wps = mpsum.tile([P, MMCOLS], F32, tag="ps")
            for _ in range(5):
                nc.tensor.matmul(wps[:, :NT], wt[:, :P], wt[:], start=True, stop=True)

            GRP = 8  # prep-group row-blocks: shortens the data->scale chain

            # Normalize `cnt` row-blocks of `raw` (slices of the preloaded
            # tiles) in groups of GRP. Returns a list of (normalized tile,
            # group size).
            def prep_stats(raw, b0, cnt):
                groups = []
                for g0 in range(0, cnt, GRP):
                    gcnt = min(GRP, cnt - g0)
                    src = raw[:, b0 + g0 : b0 + g0 + gcnt, :]
                    sq = sqp.tile([P, GRP, D], F32, tag="sq")
                    ss = statp.tile([P, GRP], F32, tag="ss")
                    # Square on GpSimd (otherwise idle; SBUF->SBUF is legal
                    # there) so ACT's duty budget is spent on PSUM copies.
                    nc.gpsimd.tensor_mul(sq[:, :gcnt, :], src, src)
                    nc.vector.reduce_sum(
                        ss[:, :gcnt], sq[:, :gcnt, :], axis=mybir.AxisListType.X
                    )
                    nrm = statp.tile([P, GRP], F32, tag="nrm")
                    nc.scalar.sqrt(nrm[:, :gcnt], ss[:, :gcnt])
                    inv = statp.tile([P, GRP], F32, tag="inv")
                    nc.vector.reciprocal(inv[:, :gcnt], nrm[:, :gcnt])
                    # One group-wide row scale (in1 free-dim-broadcast), DVE,
                    # casting to fp16 on write: halves the transpose
                    # LDWEIGHTS bytes, the PSUM traffic, and the round-copy.
                    n16 = n16p.tile([P, GRP, D], F16, tag="n16")
                    nc.vector.tensor_mul(
                        n16[:, :gcnt, :],
                        src,
                        inv[:, :gcnt].unsqueeze(2).broadcast_to((P, gcnt, D)),
                    )
                    groups.append((n16, gcnt))
                return groups

            # PE-transpose normalized groups into dstT columns (fp32r).
            # 4 transposes share one PSUM bank so the SBUF drain is one
            # activation copy per 512 columns instead of four per 128.
            def prep_transpose(groups, dstT):
                col = 0
                for sq, gcnt in groups:
                    for g in range(0, gcnt, 4):
                        qn = min(4, gcnt - g)
                        pt = tpsum.tile([P, 4 * P], F16)
                        for k in range(qn):
                            nc.tensor.transpose(
                                pt[:, k * P : (k + 1) * P], sq[:, g + k, :], ident[:]
                            )
                        # Casts fp32 -> fp16. 16-bit operands run the PE at
                        # full rate with half the LDWEIGHTS time and roughly
                        # half the PE power of fp32r -- power is what drives
                        # the HAM duty-cycle throttle that otherwise makes
                        # the PE the pipeline bottleneck. Normalized rows
                        # are in [-1,1], so fp16 (10 mantissa bits) keeps
                        # the dot-product error ~1e-3 absolute.
                        nc.scalar.copy(
                            dstT[:, col : col + qn * P], pt[:, : qn * P]
                        )
                        col += qn * P
                return col

            # x^T [d, rows_per_core], built once.
            xT = persist.tile([P, rows_per_core], F16)
            x_sq = prep_stats(xraw, 0, nbx)

            # Software-pipelined stats: chunk c+1's normalize is traced
            # before chunk c's matmul/copy phase, so on each engine FIFO the
            # prep ops run ahead of the copy flood and the PE never starves
            # waiting for the next chunk's operands.
            y_sq = {0: prep_stats(yraw, 0, chunk_cols[0] // P)}

            # x transposes after the first y-chunk's stats are in flight.
            prep_transpose(x_sq, xT[:])

            copy_rr = 0
            yTc = ytp.tile([P, OCHUNK], F16, tag="yTc")
            prep_transpose(y_sq.pop(0), yTc[:, : chunk_cols[0]])
            for c, cols in enumerate(chunk_cols):
                col0 = chunk_starts[c]
                has_next = c + 1 < len(chunk_cols)
                if has_next:
                    y_sq[c + 1] = prep_stats(
                        yraw, chunk_starts[c + 1] // P, chunk_cols[c + 1] // P
                    )
                yTc_next = None
                for i in range(nbx):
                    if i == nbx // 2 and has_next:
                        # Hoist next chunk's transposes into the middle of
                        # this chunk's matmul stream: the PE absorbs them
                        # while output copies drain, so there is no idle gap
                        # at the chunk boundary.
                        yTc_next = ytp.tile([P, OCHUNK], F16, tag="yTc")
                        prep_transpose(
                            y_sq.pop(c + 1), yTc_next[:, : chunk_cols[c + 1]]
                        )
                    lhs = xT[:, i * P : (i + 1) * P]
                    ob = obufp.tile([P, OCHUNK], F32, tag="ob")
                    for h0 in range(0, cols, MMCOLS):
                        hcols = min(MMCOLS, cols - h0)
                        ps = mpsum.tile([P, MMCOLS], F32)
                        for j in range(h0, h0 + hcols, NT):
                            nc.tensor.matmul(
                                ps[:, j - h0 : j - h0 + NT],
                                lhs,
                                yTc[:, j : j + NT],
                                start=True,
                                stop=True,
                            )
                        dst = ob[:, h0 : h0 + hcols]
                        # Balance PSUM->SBUF drain between DVE and ACT
                        # (GpSimd cannot read PSUM on TRN2).
                        if copy_rr % 2 == 0:
                            nc.vector.tensor_copy(dst, ps[:, :hcols])
                        else:
                            nc.scalar.copy(dst, ps[:, :hcols])
                        copy_rr += 1
                    nc.sync.dma_start(
                        out=out[i * P : (i + 1) * P, col0 : col0 + cols],
                        in_=ob[:, :cols],
                    )
                if has_next:
                    yTc = yTc_next

    nc.finalize()  # runs Bacc.compile(): reg alloc + event-sem wait splitting
    return nc


_NC_CACHE: dict[tuple[int, int], bass.Bass] = {}


def run_spmd(input1: np.ndarray, input2: np.ndarray, **kwargs):
    """Shard, run on 8 cores, gather. Returns (output, BassKernelResults)."""
    input1 = np.ascontiguousarray(np.asarray(input1, dtype=np.float32))
    input2 = np.ascontiguousarray(np.asarray(input2, dtype=np.float32))
    n, d = input1.shape
    m, d2 = input2.shape
    assert d == D and d2 == D and n % N_CORES == 0
    rows = n // N_CORES

    key = (rows, m)
    if key not in _NC_CACHE:
        _NC_CACHE[key] = build_nc(rows, m)
    nc = _NC_CACHE[key]

    # Partition-major pre-transpose (see build_nc): x'[p*nbx+b] = x[b*128+p],
    # y'[p*nby+b] = y[b*128+p]. Gives each SBUF partition a contiguous
    # DRAM run to load; block/column order on-device is unchanged, so the
    # output needs no unscrambling.
    nbx = rows // P
    nby = m // P
    y_pm = np.ascontiguousarray(
        input2.reshape(nby, P, D).transpose(1, 0, 2).reshape(m, D)
    )
    in_maps = [
        {
            "x": np.ascontiguousarray(
                input1[c * rows : (c + 1) * rows]
                .reshape(nbx, P, D)
                .transpose(1, 0, 2)
                .reshape(rows, D)
            ),
            "y": y_pm,
        }
        for c in range(N_CORES)
    ]
    res = run_bass_kernel_spmd(nc, in_maps, core_ids=list(range(N_CORES)), **kwargs)
    out = np.concatenate([res.results[c]["out"] for c in range(N_CORES)], axis=0)
    return out, res


def kernel(input1: np.ndarray, input2: np.ndarray) -> np.ndarray:
    return run_spmd(input1, input2)[0]


# revision 11
# speedup vs baseline: 1.1067x; 1.1067x over previous
"""Pairwise cosine similarity on 8 Trainium2 NeuronCores.

Computes sim[n, m] = <x_n, y_m> / max(||x_n|| * ||y_m||, eps) for
input1 [8192, 128], input2 [8192, 128] -> out [8192, 8192] (all fp32).

Sharding: input1 rows are split 8 ways (data parallel, 1024 rows/core);
input2 is replicated. Each core computes one [1024, 8192] output stripe;
the host concatenates stripes.

The kernel is HBM-store-bound (32 MiB of output per core vs ~27us of
matmul), so the schedule is built around keeping the Sync-HWDGE store
queue saturated at its ~400 GB/s packet-rate ceiling from as early in
the kernel as possible:

- The host feeds operands partition-major (x'[p*nbx+b] = x[b*128+p]),
  so every SBUF partition loads one contiguous DRAM run and load DMAs
  use KB-scale packets instead of one packet per 512B row.
- All loads are issued up front on ACT's HWDGE queue (separate FIFO
  from the store queue; nothing ever blocks a load issue), into
  persistent raw tiles.
- Rows of both inputs are L2-normalized in natural layout, PE-transposed
  into [d, rows] layout (rounded to fp32r), and each output stripe is a
  single matmul of the normalized operands. PSUM results stream through
  SBUF staging to DRAM in 2048-column chunks: 8 KiB store lines, the
  packet size at which the 16 DMA engines peak.

Note on eps: the reference divides by max(n1*n2, 1e-8). For these inputs
row norms are ~sqrt(128), so the eps clamp never binds and normalizing
each operand first is numerically equivalent (to fp32 rounding).
"""

import numpy as np

import concourse.bass as bass
import concourse.tile as tile
from concourse import bacc, masks, mybir
from concourse.bass_utils import run_bass_kernel_spmd

N_CORES = 8
D = 128          # feature dim == partition count
P = 128          # SBUF partitions
NT = 512         # matmul moving free dim (one fp32 PSUM bank)
OCHUNK = 2048    # output columns per staging buffer (8KB/partition, 1MiB DMA)
MMCOLS = 1024    # PSUM matmul tile columns (2 banks, 2 matmuls, 1 copy)

F32 = mybir.dt.float32
F16 = mybir.dt.float16
BF16 = mybir.dt.bfloat16
ACTF = mybir.ActivationFunctionType


def build_nc(rows_per_core: int, corpus_rows: int) -> bass.Bass:
    # Bacc (not raw Bass): its compile() pipeline splits multi-sem waits into
    # event-semaphore instructions, which self-loading fp32/fp32r matmuls
    # need (the ISA LDWEIGHTS struct can carry only one wait).
    nc = bacc.Bacc(None)

    x = nc.dram_tensor("x", [rows_per_core, D], F32, kind="ExternalInput")
    y = nc.dram_tensor("y", [corpus_rows, D], F32, kind="ExternalInput")
    out = nc.dram_tensor(
        "out", [rows_per_core, corpus_rows], F32, kind="ExternalOutput"
    )

    nbx = rows_per_core // P         # x row-blocks (8)
    nby = corpus_rows // P           # y row-blocks (64)

    with tile.TileContext(nc) as tc:
        with (
            tc.tile_pool(name="const", bufs=1) as constp,
            tc.tile_pool(name="persist", bufs=1) as persist,
            tc.tile_pool(name="yt", bufs=3) as ytp,
            tc.tile_pool(name="stat", bufs=6) as statp,
            tc.tile_pool(name="sq", bufs=5) as sqp,
            tc.tile_pool(name="n16", bufs=5) as n16p,
            tc.tile_pool(name="obuf", bufs=4) as obufp,
            tc.tile_pool(name="tp", bufs=2, space=bass.MemorySpace.PSUM) as tpsum,
            tc.tile_pool(name="mm", bufs=3, space=bass.MemorySpace.PSUM) as mpsum,
        ):
            ident = constp.tile([P, P], F16)
            masks.make_identity(nc, ident[:])

            # Partition-major DRAM layout (host pre-transposes the shards):
            # x row b*128+p lives at [p, b, :], so partition p's rows are one
            # contiguous nbx*512B run. Block b still holds rows {b*128+p}, so
            # xT/yT column order (and therefore the stores) stays natural.
            x_view = x[:].rearrange("(p b) d -> p b d", p=P)
            y_view = y[:].rearrange("(p b) d -> p b d", p=P)

            # Uniform max-size chunks: 8KB store lines are the packet size at
            # which the 16 DMA engines hit their ~400GB/s aggregate ceiling
            # (smaller chunks pay 112ns/packet overhead on less data).
            chunk_cols = [OCHUNK] * (corpus_rows // OCHUNK)
            rem = corpus_rows - sum(chunk_cols)
            if rem:
                chunk_cols.append(rem)
            chunk_starts = []
            s = 0
            for cols in chunk_cols:
                chunk_starts.append(s)
                s += cols

            # All loads issued up front on ACT's HWDGE queue: its FIFO holds
            # only dma_starts at this point, so every transfer is in flight
            # within the first microseconds regardless of compute stalls.
            # (SWDGE's GpSimd firmware needs ~9.5us for its first packet;
            # the Sync FIFO would head-of-line-block stores behind loads.)
            xraw = persist.tile([P, nbx, D], F32)
            yraw = persist.tile([P, nby, D], F32)
            # x + first y chunk ride the Sync queue: Sync clears its startup
            # barrier ~2us before ACT, and only 1.5MB sits ahead of the first
            # store descriptor. Later chunks go on ACT's queue so the store
            # stream never queues behind them.
            nc.sync.dma_start(out=xraw[:], in_=x_view[:, :nbx, :])
            for c, cols in enumerate(chunk_cols):
                b0 = chunk_starts[c] // P
                bcnt = cols // P
                eng = nc.sync if c == 0 else nc.scalar
                eng.dma_start(
                    out=yraw[:, b0 : b0 + bcnt, :],
                    in_=y_view[:, b0 : b0 + bcnt, :],
                )

            # PE warm-up: dummy bf16 matmuls overlapping the initial load so
            # the HAM clock gate opens (1.2 -> 2.4 GHz) before the first real
            # matmul.
            wt = constp.tile([P, NT], BF16)
            # Memset on DVE: GpSimd's first op lands ~6us in (Q7 launch),
            # which would gate the first warm-up matmul.
            nc.vector.memset(wt[:], 0.0)
            wps = mpsum.tile([P, MMCOLS], F32, tag="ps")
            for _ in range(8):
                nc.tensor.matmul(wps[:, :NT], wt[:, :P], wt[:], start=True, stop=True)

            GRP = 8  # prep-group row-blocks: shortens the data->scale chain

            # Normalize `cnt` row-blocks of `raw` (slices of the preloaded
            # tiles) in groups of GRP. Returns a list of (normalized tile,
            # group size).
            def prep_stats(raw, b0, cnt):
                groups = []
                for g0 in range(0, cnt, GRP):
                    gcnt = min(GRP, cnt - g0)
                    src = raw[:, b0 + g0 : b0 + g0 + gcnt, :]
                    sq = sqp.tile([P, GRP, D], F32, tag="sq")
                    ss = statp.tile([P, GRP], F32, tag="ss")
                    nc.scalar.square(sq[:, :gcnt, :], src)
                    nc.vector.reduce_sum(
                        ss[:, :gcnt], sq[:, :gcnt, :], axis=mybir.AxisListType.X
                    )
                    nrm = statp.tile([P, GRP], F32, tag="nrm")
                    nc.scalar.sqrt(nrm[:, :gcnt], ss[:, :gcnt])
                    inv = statp.tile([P, GRP], F32, tag="inv")
                    nc.vector.reciprocal(inv[:, :gcnt], nrm[:, :gcnt])
                    # One group-wide row scale (in1 free-dim-broadcast), DVE,
                    # casting to fp16 on write: halves the transpose
                    # LDWEIGHTS bytes, the PSUM traffic, and the round-copy.
                    n16 = n16p.tile([P, GRP, D], F16, tag="n16")
                    nc.vector.tensor_mul(
                        n16[:, :gcnt, :],
                        src,
                        inv[:, :gcnt].unsqueeze(2).broadcast_to((P, gcnt, D)),
                    )
                    groups.append((n16, gcnt))
                return groups

            # PE-transpose normalized groups into dstT columns (fp32r).
            # 4 transposes share one PSUM bank so the SBUF drain is one
            # activation copy per 512 columns instead of four per 128.
            def prep_transpose(groups, dstT):
                col = 0
                for sq, gcnt in groups:
                    for g in range(0, gcnt, 4):
                        qn = min(4, gcnt - g)
                        pt = tpsum.tile([P, 4 * P], F16)
                        for k in range(qn):
                            nc.tensor.transpose(
                                pt[:, k * P : (k + 1) * P], sq[:, g + k, :], ident[:]
                            )
                        # Casts fp32 -> fp16. 16-bit operands run the PE at
                        # full rate with half the LDWEIGHTS time and roughly
                        # half the PE power of fp32r -- power is what drives
                        # the HAM duty-cycle throttle that otherwise makes
                        # the PE the pipeline bottleneck. Normalized rows
                        # are in [-1,1], so fp16 (10 mantissa bits) keeps
                        # the dot-product error ~1e-3 absolute.
                        nc.scalar.copy(
                            dstT[:, col : col + qn * P], pt[:, : qn * P]
                        )
                        col += qn * P
                return col

            # x^T [d, rows_per_core], built once.
            xT = persist.tile([P, rows_per_core], F16)
            x_sq = prep_stats(xraw, 0, nbx)

            # Software-pipelined stats: chunk c+1's normalize is traced
            # before chunk c's matmul/copy phase, so on each engine FIFO the
            # prep ops run ahead of the copy flood and the PE never starves
            # waiting for the next chunk's operands.
            y_sq = {0: prep_stats(yraw, 0, chunk_cols[0] // P)}

            # x transposes after the first y-chunk's stats are in flight.
            prep_transpose(x_sq, xT[:])

            copy_rr = 0
            yTc = ytp.tile([P, OCHUNK], F16, tag="yTc")
            prep_transpose(y_sq.pop(0), yTc[:, : chunk_cols[0]])
            for c, cols in enumerate(chunk_cols):
                col0 = chunk_starts[c]
                has_next = c + 1 < len(chunk_cols)
                if has_next:
                    y_sq[c + 1] = prep_stats(
                        yraw, chunk_starts[c + 1] // P, chunk_cols[c + 1] // P
                    )
                yTc_next = None
                for i in range(nbx):
                    if i == nbx // 2 and has_next:
                        # Hoist next chunk's transposes into the middle of
                        # this chunk's matmul stream: the PE absorbs them
                        # while output copies drain, so there is no idle gap
                        # at the chunk boundary.
                        yTc_next = ytp.tile([P, OCHUNK], F16, tag="yTc")
                        prep_transpose(
                            y_sq.pop(c + 1), yTc_next[:, : chunk_cols[c + 1]]
                        )
                    lhs = xT[:, i * P : (i + 1) * P]
                    ob = obufp.tile([P, OCHUNK], F32, tag="ob")
                    for h0 in range(0, cols, MMCOLS):
                        hcols = min(MMCOLS, cols - h0)
                        ps = mpsum.tile([P, MMCOLS], F32)
                        for j in range(h0, h0 + hcols, NT):
                            nc.tensor.matmul(
                                ps[:, j - h0 : j - h0 + NT],
                                lhs,
                                yTc[:, j : j + NT],
                                start=True,
                                stop=True,
                            )
                        dst = ob[:, h0 : h0 + hcols]
                        # Balance PSUM->SBUF drain between DVE and ACT
                        # (GpSimd cannot read PSUM on TRN2).
                        if copy_rr % 2 == 0:
                            nc.vector.tensor_copy(dst, ps[:, :hcols])
                        else:
                            nc.scalar.copy(dst, ps[:, :hcols])
                        copy_rr += 1
                    nc.sync.dma_start(
                        out=out[i * P : (i + 1) * P, col0 : col0 + cols],
                        in_=ob[:, :cols],
                    )
                if has_next:
                    yTc = yTc_next

    nc.finalize()  # runs Bacc.compile(): reg alloc + event-sem wait splitting
    return nc


_NC_CACHE: dict[tuple[int, int], bass.Bass] = {}


def run_spmd(input1: np.ndarray, input2: np.ndarray, **kwargs):
    """Shard, run on 8 cores, gather. Returns (output, BassKernelResults)."""
    input1 = np.ascontiguousarray(np.asarray(input1, dtype=np.float32))
    input2 = np.ascontiguousarray(np.asarray(input2, dtype=np.float32))
    n, d = input1.shape
    m, d2 = input2.shape
    assert d == D and d2 == D and n % N_CORES == 0
    rows = n // N_CORES

    key = (rows, m)
    if key not in _NC_CACHE:
        _NC_CACHE[key] = build_nc(rows, m)
    nc = _NC_CACHE[key]

    # Partition-major pre-transpose (see build_nc): x'[p*nbx+b] = x[b*128+p],
    # y'[p*nby+b] = y[b*128+p]. Gives each SBUF partition a contiguous
    # DRAM run to load; block/column order on-device is unchanged, so the
    # output needs no unscrambling.
    nbx = rows // P
    nby = m // P
    y_pm = np.ascontiguousarray(
        input2.reshape(nby, P, D).transpose(1, 0, 2).reshape(m, D)
    )
    in_maps = [
        {
            "x": np.ascontiguousarray(
                input1[c * rows : (c + 1) * rows]
                .reshape(nbx, P, D)
                .transpose(1, 0, 2)
                .reshape(rows, D)
            ),
            "y": y_pm,
        }
        for c in range(N_CORES)
    ]
    res = run_bass_kernel_spmd(nc, in_maps, core_ids=list(range(N_CORES)), **kwargs)
    out = np.concatenate([res.results[c]["out"] for c in range(N_CORES)], axis=0)
    return out, res


def kernel(input1: np.ndarray, input2: np.ndarray) -> np.ndarray:
    return run_spmd(input1, input2)[0]


# revision 12
# speedup vs baseline: 1.1343x; 1.0250x over previous
"""Pairwise cosine similarity on 8 Trainium2 NeuronCores.

Computes sim[n, m] = <x_n, y_m> / max(||x_n|| * ||y_m||, eps) for
input1 [8192, 128], input2 [8192, 128] -> out [8192, 8192] (all fp32).

Sharding: input1 rows are split 8 ways (data parallel, 1024 rows/core);
input2 is replicated. Each core computes one [1024, 8192] output stripe;
the host concatenates stripes.

The kernel is HBM-store-bound (32 MiB of output per core vs ~27us of
matmul), so the schedule is built around keeping the Sync-HWDGE store
queue saturated at its ~400 GB/s packet-rate ceiling from as early in
the kernel as possible:

- The host feeds operands partition-major (x'[p*nbx+b] = x[b*128+p]),
  so every SBUF partition loads one contiguous DRAM run and load DMAs
  use KB-scale packets instead of one packet per 512B row.
- All loads are issued up front on ACT's HWDGE queue (separate FIFO
  from the store queue; nothing ever blocks a load issue), into
  persistent raw tiles.
- Rows of both inputs are L2-normalized in natural layout, PE-transposed
  into [d, rows] layout (rounded to fp32r), and each output stripe is a
  single matmul of the normalized operands. PSUM results stream through
  SBUF staging to DRAM in 2048-column chunks: 8 KiB store lines, the
  packet size at which the 16 DMA engines peak.

Note on eps: the reference divides by max(n1*n2, 1e-8). For these inputs
row norms are ~sqrt(128), so the eps clamp never binds and normalizing
each operand first is numerically equivalent (to fp32 rounding).
"""

import numpy as np

import concourse.bass as bass
import concourse.tile as tile
from concourse import bacc, masks, mybir
from concourse.bass_utils import run_bass_kernel_spmd

N_CORES = 8
D = 128          # feature dim == partition count
P = 128          # SBUF partitions
NT = 512         # matmul moving free dim (one fp32 PSUM bank)
OCHUNK = 2048    # output columns per staging buffer (8KB/partition, 1MiB DMA)
MMCOLS = 1024    # PSUM matmul tile columns (2 banks, 2 matmuls, 1 copy)

F32 = mybir.dt.float32
F16 = mybir.dt.float16
BF16 = mybir.dt.bfloat16
ACTF = mybir.ActivationFunctionType


def build_nc(rows_per_core: int, corpus_rows: int) -> bass.Bass:
    # Bacc (not raw Bass): its compile() pipeline splits multi-sem waits into
    # event-semaphore instructions, which self-loading fp32/fp32r matmuls
    # need (the ISA LDWEIGHTS struct can carry only one wait).
    nc = bacc.Bacc(None)

    x = nc.dram_tensor("x", [rows_per_core, D], F32, kind="ExternalInput")
    y = nc.dram_tensor("y", [corpus_rows, D], F32, kind="ExternalInput")
    out = nc.dram_tensor(
        "out", [rows_per_core, corpus_rows], F32, kind="ExternalOutput"
    )

    nbx = rows_per_core // P         # x row-blocks (8)
    nby = corpus_rows // P           # y row-blocks (64)

    with tile.TileContext(nc) as tc:
        with (
            tc.tile_pool(name="const", bufs=1) as constp,
            tc.tile_pool(name="persist", bufs=1) as persist,
            tc.tile_pool(name="yt", bufs=3) as ytp,
            tc.tile_pool(name="stat", bufs=6) as statp,
            tc.tile_pool(name="sq", bufs=5) as sqp,
            tc.tile_pool(name="n16", bufs=5) as n16p,
            tc.tile_pool(name="obuf", bufs=4) as obufp,
            tc.tile_pool(name="tp", bufs=2, space=bass.MemorySpace.PSUM) as tpsum,
            tc.tile_pool(name="mm", bufs=3, space=bass.MemorySpace.PSUM) as mpsum,
        ):
            ident = constp.tile([P, P], F16)
            masks.make_identity(nc, ident[:])

            # Partition-major DRAM layout (host pre-transposes the shards):
            # x row b*128+p lives at [p, b, :], so partition p's rows are one
            # contiguous nbx*512B run. Block b still holds rows {b*128+p}, so
            # xT/yT column order (and therefore the stores) stays natural.
            x_view = x[:].rearrange("(p b) d -> p b d", p=P)
            y_view = y[:].rearrange("(p b) d -> p b d", p=P)

            # Graduated chunk schedule: the first stores are gated on
            # chunk0's full normalize+transpose+matmul chain running at the
            # cold 1.2GHz clock, so a small chunk0 starts the store stream
            # ~10us earlier; mid-stream chunks are 2048 cols (8KB lines,
            # the packet size at which the 16 DMA engines peak ~406GB/s);
            # a small last chunk shortens the drain after the final matmul.
            if corpus_rows == 8192:
                chunk_cols = [512, 1024, 1536, 2048, 2048, 1024]
            else:
                chunk_cols = [OCHUNK] * (corpus_rows // OCHUNK)
                rem = corpus_rows - sum(chunk_cols)
                if rem:
                    chunk_cols.append(rem)
            assert sum(chunk_cols) == corpus_rows
            chunk_starts = []
            s = 0
            for cols in chunk_cols:
                chunk_starts.append(s)
                s += cols

            # All loads issued up front on ACT's HWDGE queue: its FIFO holds
            # only dma_starts at this point, so every transfer is in flight
            # within the first microseconds regardless of compute stalls.
            # (SWDGE's GpSimd firmware needs ~9.5us for its first packet;
            # the Sync FIFO would head-of-line-block stores behind loads.)
            xraw = persist.tile([P, nbx, D], F32)
            yraw = persist.tile([P, nby, D], F32)
            # x + first y chunk ride the Sync queue: Sync clears its startup
            # barrier ~2us before ACT, and only 1.5MB sits ahead of the first
            # store descriptor. Later chunks go on ACT's queue so the store
            # stream never queues behind them.
            nc.sync.dma_start(out=xraw[:], in_=x_view[:, :nbx, :])
            for c, cols in enumerate(chunk_cols):
                b0 = chunk_starts[c] // P
                bcnt = cols // P
                eng = nc.sync if c == 0 else nc.scalar
                eng.dma_start(
                    out=yraw[:, b0 : b0 + bcnt, :],
                    in_=y_view[:, b0 : b0 + bcnt, :],
                )

            # PE warm-up: dummy bf16 matmuls overlapping the initial load so
            # the HAM clock gate opens (1.2 -> 2.4 GHz) before the first real
            # matmul.
            wt = constp.tile([P, NT], BF16)
            # Memset on DVE: GpSimd's first op lands ~6us in (Q7 launch),
            # which would gate the first warm-up matmul.
            nc.vector.memset(wt[:], 0.0)
            wps = mpsum.tile([P, MMCOLS], F32, tag="ps")
            # 5 dummies bridge PE from engine-start (~8us with LDW) to the
            # first transposes; real transposes + matmuls then keep the PE
            # stream continuous so the HAM clock gate opens during real work.
            for _ in range(5):
                nc.tensor.matmul(wps[:, :NT], wt[:, :P], wt[:], start=True, stop=True)

            GRP = 8  # prep-group row-blocks: shortens the data->scale chain

            # Normalize `cnt` row-blocks of `raw` (slices of the preloaded
            # tiles) in groups of GRP. Returns a list of (normalized tile,
            # group size).
            def prep_stats(raw, b0, cnt):
                groups = []
                for g0 in range(0, cnt, GRP):
                    gcnt = min(GRP, cnt - g0)
                    src = raw[:, b0 + g0 : b0 + g0 + gcnt, :]
                    sq = sqp.tile([P, GRP, D], F32, tag="sq")
                    ss = statp.tile([P, GRP], F32, tag="ss")
                    nc.scalar.square(sq[:, :gcnt, :], src)
                    nc.vector.reduce_sum(
                        ss[:, :gcnt], sq[:, :gcnt, :], axis=mybir.AxisListType.X
                    )
                    nrm = statp.tile([P, GRP], F32, tag="nrm")
                    nc.scalar.sqrt(nrm[:, :gcnt], ss[:, :gcnt])
                    inv = statp.tile([P, GRP], F32, tag="inv")
                    nc.vector.reciprocal(inv[:, :gcnt], nrm[:, :gcnt])
                    # One group-wide row scale (in1 free-dim-broadcast), DVE,
                    # casting to fp16 on write: halves the transpose
                    # LDWEIGHTS bytes, the PSUM traffic, and the round-copy.
                    n16 = n16p.tile([P, GRP, D], F16, tag="n16")
                    nc.vector.tensor_mul(
                        n16[:, :gcnt, :],
                        src,
                        inv[:, :gcnt].unsqueeze(2).broadcast_to((P, gcnt, D)),
                    )
                    groups.append((n16, gcnt))
                return groups

            # PE-transpose normalized groups into dstT columns (fp32r).
            # 4 transposes share one PSUM bank so the SBUF drain is one
            # activation copy per 512 columns instead of four per 128.
            def prep_transpose(groups, dstT):
                col = 0
                for sq, gcnt in groups:
                    for g in range(0, gcnt, 4):
                        qn = min(4, gcnt - g)
                        pt = tpsum.tile([P, 4 * P], F16)
                        for k in range(qn):
                            nc.tensor.transpose(
                                pt[:, k * P : (k + 1) * P], sq[:, g + k, :], ident[:]
                            )
                        # Casts fp32 -> fp16. 16-bit operands run the PE at
                        # full rate with half the LDWEIGHTS time and roughly
                        # half the PE power of fp32r -- power is what drives
                        # the HAM duty-cycle throttle that otherwise makes
                        # the PE the pipeline bottleneck. Normalized rows
                        # are in [-1,1], so fp16 (10 mantissa bits) keeps
                        # the dot-product error ~1e-3 absolute.
                        nc.scalar.copy(
                            dstT[:, col : col + qn * P], pt[:, : qn * P]
                        )
                        col += qn * P
                return col

            # x^T [d, rows_per_core], built once.
            xT = persist.tile([P, rows_per_core], F16)
            x_sq = prep_stats(xraw, 0, nbx)

            # Software-pipelined stats: chunk c+1's normalize is traced
            # before chunk c's matmul/copy phase, so on each engine FIFO the
            # prep ops run ahead of the copy flood and the PE never starves
            # waiting for the next chunk's operands.
            y_sq = {0: prep_stats(yraw, 0, chunk_cols[0] // P)}

            # x transposes after the first y-chunk's stats are in flight.
            prep_transpose(x_sq, xT[:])

            copy_rr = 0
            yTc = ytp.tile([P, OCHUNK], F16, tag="yTc")
            prep_transpose(y_sq.pop(0), yTc[:, : chunk_cols[0]])
            for c, cols in enumerate(chunk_cols):
                col0 = chunk_starts[c]
                has_next = c + 1 < len(chunk_cols)
                if has_next:
                    y_sq[c + 1] = prep_stats(
                        yraw, chunk_starts[c + 1] // P, chunk_cols[c + 1] // P
                    )
                yTc_next = None
                for i in range(nbx):
                    if i == nbx // 2 and has_next:
                        # Hoist next chunk's transposes into the middle of
                        # this chunk's matmul stream: the PE absorbs them
                        # while output copies drain, so there is no idle gap
                        # at the chunk boundary.
                        yTc_next = ytp.tile([P, OCHUNK], F16, tag="yTc")
                        prep_transpose(
                            y_sq.pop(c + 1), yTc_next[:, : chunk_cols[c + 1]]
                        )
                    lhs = xT[:, i * P : (i + 1) * P]
                    ob = obufp.tile([P, OCHUNK], F32, tag="ob")
                    for h0 in range(0, cols, MMCOLS):
                        hcols = min(MMCOLS, cols - h0)
                        ps = mpsum.tile([P, MMCOLS], F32)
                        for j in range(h0, h0 + hcols, NT):
                            nc.tensor.matmul(
                                ps[:, j - h0 : j - h0 + NT],
                                lhs,
                                yTc[:, j : j + NT],
                                start=True,
                                stop=True,
                            )
                        dst = ob[:, h0 : h0 + hcols]
                        # Balance PSUM->SBUF drain between DVE and ACT
                        # (GpSimd cannot read PSUM on TRN2).
                        if copy_rr % 2 == 0:
                            nc.vector.tensor_copy(dst, ps[:, :hcols])
                        else:
                            nc.scalar.copy(dst, ps[:, :hcols])
                        copy_rr += 1
                    nc.sync.dma_start(
                        out=out[i * P : (i + 1) * P, col0 : col0 + cols],
                        in_=ob[:, :cols],
                    )
                if has_next:
                    yTc = yTc_next

    nc.finalize()  # runs Bacc.compile(): reg alloc + event-sem wait splitting
    return nc


_NC_CACHE: dict[tuple[int, int], bass.Bass] = {}


def run_spmd(input1: np.ndarray, input2: np.ndarray, **kwargs):
    """Shard, run on 8 cores, gather. Returns (output, BassKernelResults)."""
    input1 = np.ascontiguousarray(np.asarray(input1, dtype=np.float32))
    input2 = np.ascontiguousarray(np.asarray(input2, dtype=np.float32))
    n, d = input1.shape
    m, d2 = input2.shape
    assert d == D and d2 == D and n % N_CORES == 0
    rows = n // N_CORES

    key = (rows, m)
    if key not in _NC_CACHE:
        _NC_CACHE[key] = build_nc(rows, m)
    nc = _NC_CACHE[key]

    # Partition-major pre-transpose (see build_nc): x'[p*nbx+b] = x[b*128+p],
    # y'[p*nby+b] = y[b*128+p]. Gives each SBUF partition a contiguous
    # DRAM run to load; block/column order on-device is unchanged, so the
    # output needs no unscrambling.
    nbx = rows // P
    nby = m // P
    y_pm = np.ascontiguousarray(
        input2.reshape(nby, P, D).transpose(1, 0, 2).reshape(m, D)
    )
    in_maps = [
        {
            "x": np.ascontiguousarray(
                input1[c * rows : (c + 1) * rows]
                .reshape(nbx, P, D)
                .transpose(1, 0, 2)
                .reshape(rows, D)
            ),
            "y": y_pm,
        }
        for c in range(N_CORES)
    ]
    res = run_bass_kernel_spmd(nc, in_maps, core_ids=list(range(N_CORES)), **kwargs)
    out = np.concatenate([res.results[c]["out"] for c in range(N_CORES)], axis=0)
    return out, res


def kernel(input1: np.ndarray, input2: np.ndarray) -> np.ndarray:
    return run_spmd(input1, input2)[0]
